# revision 3
# baseline (speedup 1.0000x reference)
"""Trainium2 Bass kernel for nn_MultiHeadAttention (B=2, S=2048, H=1024, 16 heads).

Sharding: tensor-parallel over heads — 2 heads per core on 8 cores.
Per core: Q/K/V projections for its 128 features, dual score matmuls
(S[i,j] for the attention-weights output + softmax sums via accum_out;
S.T[j,i] recomputed so exp(S.T) feeds the context matmul as the moving
operand — avoids PE transposes entirely), context as ctx.T per head,
out-projection per head with the softmax normalization folded into the
final per-partition combine.  Host gathers attn heads and sums partials.

Self-contained: hardcodes shapes; needs only concourse + numpy + ml_dtypes.
"""
import numpy as np
import ml_dtypes

import concourse.bacc as bacc
import concourse.tile as tile
import concourse.mybir as mybir
from concourse.bass import ts
from concourse.bass_utils import run_bass_kernel_spmd
from concourse.masks import make_identity

B, S, HID = 2, 2048, 1024
NCORES, NH, DH = 8, 16, 64
HPC = NH // NCORES          # 2 heads per core
F = HPC * DH                # 128 features per core
BS = B * S                  # 4096
NI = S // 128               # 16 row chunks per batch
NJ = S // 128               # 16 col chunks per batch
NC8 = HID // 128            # 8 contraction chunks for projections

BF16 = mybir.dt.bfloat16
F32 = mybir.dt.float32
AF = mybir.ActivationFunctionType
ALU = mybir.AluOpType
BF = ml_dtypes.bfloat16


def build_nc():
    nc = bacc.Bacc("TRN2", target_bir_lowering=False, debug=False,
                   num_devices=NCORES)
    xT = nc.dram_tensor("xT", [HID, BS], BF16, kind="ExternalInput")
    wqT = nc.dram_tensor("wqT", [HID, F], BF16, kind="ExternalInput")
    wkT = nc.dram_tensor("wkT", [HID, F], BF16, kind="ExternalInput")
    wvT = nc.dram_tensor("wvT", [HID, F], BF16, kind="ExternalInput")
    woT = nc.dram_tensor("woT", [F, HID], BF16, kind="ExternalInput")
    bq = nc.dram_tensor("bq", [F, 1], F32, kind="ExternalInput")
    bk = nc.dram_tensor("bk", [F, 1], F32, kind="ExternalInput")
    bv = nc.dram_tensor("bv", [F, 1], F32, kind="ExternalInput")
    attn_o = nc.dram_tensor("attn_o", [B, HPC, S, S], F32, kind="ExternalOutput")
    out_p = nc.dram_tensor("out_p", [BS, HID], F32, kind="ExternalOutput")

    with tile.TileContext(nc) as tc:
        with tc.tile_pool(name="consts", bufs=1) as consts, \
             tc.tile_pool(name="p1", bufs=2, space="PSUM") as p1, \
             tc.tile_pool(name="p2", bufs=2, space="PSUM") as p2, \
             tc.tile_pool(name="sb_eS", bufs=3) as sb_eS, \
             tc.tile_pool(name="sb_eT", bufs=3) as sb_eT, \
             tc.tile_pool(name="sb_at", bufs=3) as sb_at, \
             tc.tile_pool(name="sb_ct", bufs=3) as sb_ct, \
             tc.tile_pool(name="sb_out", bufs=3) as sb_out, \
             tc.tile_pool(name="sb_small", bufs=16) as sb_small:

            ident = consts.tile([128, 128], BF16)
            make_identity(nc, ident[:])

            # ---- load inputs ----
            xT_sb = consts.tile([128, NC8, BS], BF16)
            nc.sync.dma_start(out=xT_sb[:],
                              in_=xT[:, :].rearrange("(c p) i -> p c i", p=128))
            wq_sb = consts.tile([128, NC8, F], BF16)
            nc.sync.dma_start(out=wq_sb[:],
                              in_=wqT[:, :].rearrange("(c p) f -> p c f", p=128))
            wk_sb = consts.tile([128, NC8, F], BF16)
            nc.sync.dma_start(out=wk_sb[:],
                              in_=wkT[:, :].rearrange("(c p) f -> p c f", p=128))
            wv_sb = consts.tile([128, NC8, F], BF16)
            nc.sync.dma_start(out=wv_sb[:],
                              in_=wvT[:, :].rearrange("(c p) f -> p c f", p=128))
            # per-head slices of Wo^T rows, each loaded at partition base 0
            wo_h = []
            for lh in range(HPC):
                w = consts.tile([DH, HID], BF16, tag=f"wo{lh}")
                nc.sync.dma_start(out=w[:, :], in_=woT[lh * DH:(lh + 1) * DH, :])
                wo_h.append(w)
            bq_sb = consts.tile([128, 1], F32)
            nc.sync.dma_start(out=bq_sb[:], in_=bq[:, :])
            bk_sb = consts.tile([128, 1], F32)
            nc.sync.dma_start(out=bk_sb[:], in_=bk[:, :])
            bv_sb = consts.tile([128, 1], F32)
            nc.sync.dma_start(out=bv_sb[:], in_=bv[:, :])

            # ---- projections: qT/kT/vT [f=128, i=4096] bf16 with bias ----
            qT_sb = consts.tile([128, BS], BF16)
            kT_sb = consts.tile([128, BS], BF16)
            vT_sb = consts.tile([128, BS], BF16)
            for w_sb, b_sb, dest in ((wq_sb, bq_sb, qT_sb),
                                     (wk_sb, bk_sb, kT_sb),
                                     (wv_sb, bv_sb, vT_sb)):
                for ib in range(BS // 512):
                    ps = p1.tile([128, 512], F32, tag="p1")
                    for cc in range(NC8):
                        nc.tensor.matmul(ps[:],
                                         lhsT=w_sb[:, cc, :],
                                         rhs=xT_sb[:, cc, ts(ib, 512)],
                                         start=(cc == 0), stop=(cc == NC8 - 1))
                    nc.vector.tensor_scalar_add(dest[:, ts(ib, 512)], ps[:], b_sb[:])

            # ---- V into [j, f] layout via PE transpose ----
            v_all = consts.tile([128, B, NJ, F], BF16)
            for b in range(B):
                for jc in range(NJ):
                    pt = p2.tile([128, 128], BF16, tag="p2")
                    nc.tensor.transpose(pt[:], vT_sb[:, b * S + jc * 128:
                                                     b * S + (jc + 1) * 128], ident[:])
                    nc.scalar.copy(v_all[:, b, jc, :], pt[:])

            recip_all = consts.tile([128, HPC, NI], F32)

            # ---- main: per (b, lh): interleaved S.T+context and S phases ----
            for b in range(B):
                ctxT = []  # per-head unnormalized ctx.T [64, 2048] bf16
                for lh in range(HPC):
                    d0 = lh * DH
                    ct_h = [p2.tile([64, 1024], F32, tag="p2", name=f"ct_{b}_{lh}_{ih}")
                            for ih in range(2)]
                    for step in range(NJ):
                        # --- S.T for j-chunk `step`: [j=128, i=2048] ---
                        jc = step
                        eT = sb_eT.tile([128, S], BF16, tag="eT")
                        for ih in range(2):
                            ps = p1.tile([128, 1024], F32, tag="p1")
                            for iq in range(2):
                                i0 = b * S + ih * 1024 + iq * 512
                                nc.tensor.matmul(
                                    ps[:, ts(iq, 512)],
                                    lhsT=kT_sb[d0:d0 + DH,
                                               b * S + jc * 128:b * S + (jc + 1) * 128],
                                    rhs=qT_sb[d0:d0 + DH, i0:i0 + 512],
                                    start=True, stop=True)
                            nc.scalar.activation(out=eT[:, ts(ih, 1024)], in_=ps[:],
                                                 func=AF.Exp)
                        # --- context accumulation: ct[d, i] += V_jc^T-style ---
                        for ih in range(2):
                            for iq in range(2):
                                nc.tensor.matmul(
                                    ct_h[ih][:, ts(iq, 512)],
                                    lhsT=v_all[:, b, jc, d0:d0 + DH],
                                    rhs=eT[:, ih * 1024 + iq * 512:
                                           ih * 1024 + (iq + 1) * 512],
                                    start=(jc == 0), stop=(jc == NJ - 1))
                        # --- S for i-chunk `step`: [i=128, j=2048] ---
                        ic = step
                        i0 = b * S + ic * 128
                        eS = sb_eS.tile([128, S], BF16, tag="eS")
                        sume = sb_small.tile([128, 2], F32, tag="s2")
                        for jh in range(2):
                            ps = p1.tile([128, 1024], F32, tag="p1")
                            for jq in range(2):
                                j0 = b * S + jh * 1024 + jq * 512
                                nc.tensor.matmul(
                                    ps[:, ts(jq, 512)],
                                    lhsT=qT_sb[d0:d0 + DH, i0:i0 + 128],
                                    rhs=kT_sb[d0:d0 + DH, j0:j0 + 512],
                                    start=True, stop=True)
                            nc.scalar.activation(out=eS[:, ts(jh, 1024)], in_=ps[:],
                                                 func=AF.Exp,
                                                 accum_out=sume[:, jh:jh + 1])
                        sumx = sb_small.tile([128, 1], F32, tag="s1")
                        nc.vector.tensor_add(sumx[:], sume[:, 0:1], sume[:, 1:2])
                        nc.vector.reciprocal(recip_all[:, lh, ic:ic + 1], sumx[:])
                        # attn tile normalize (f32 staging): DVE half + GpSimd half
                        at = sb_at.tile([128, S], F32, tag="at")
                        nc.vector.tensor_scalar_mul(at[:, 0:1024], eS[:, 0:1024],
                                                    recip_all[:, lh, ic:ic + 1])
                        nc.gpsimd.tensor_scalar_mul(at[:, 1024:2048], eS[:, 1024:2048],
                                                    recip_all[:, lh, ic:ic + 1])
                        nc.sync.dma_start(
                            out=attn_o[b, lh, ic * 128:(ic + 1) * 128, :],
                            in_=at[:])
                    # ct -> SBUF bf16 (unnormalized ctx.T)
                    cT = sb_ct.tile([64, S], BF16, tag="ct")
                    for ih in range(2):
                        nc.vector.tensor_copy(cT[:, ts(ih, 1024)], ct_h[ih][:])
                    ctxT.append(cT)

                # ---- out-proj for batch b, fold in 1/sumexp per head ----
                for ic in range(NI):
                    i0 = b * S + ic * 128
                    po = []
                    for lh in range(HPC):
                        p = p1.tile([128, 1024], F32, tag="p1")
                        for oh in range(2):
                            nc.tensor.matmul(
                                p[:, ts(oh, 512)],
                                lhsT=ctxT[lh][:, ic * 128:(ic + 1) * 128],
                                rhs=wo_h[lh][:, ts(oh, 512)],
                                start=True, stop=True)
                        po.append(p)
                    ot = sb_out.tile([128, 1024], F32, tag="o")
                    tmp = sb_out.tile([128, 1024], F32, tag="tmp")
                    nc.vector.tensor_scalar_mul(tmp[:], po[0][:],
                                                recip_all[:, 0, ic:ic + 1])
                    nc.vector.scalar_tensor_tensor(ot[:], po[1][:],
                                                   recip_all[:, 1, ic:ic + 1],
                                                   tmp[:],
                                                   ALU.mult, ALU.add)
                    nc.sync.dma_start(out=out_p[i0:i0 + 128, :], in_=ot[:])

    nc.finalize()
    return nc


_NC = None


def _get_nc():
    global _NC
    if _NC is None:
        _NC = build_nc()
    return _NC


def kernel(query, Wq, bq, Wk, bk, Wv, bv, Wo, bo):
    query = np.asarray(query, dtype=np.float32)
    Wq = np.asarray(Wq, dtype=np.float32)
    Wk = np.asarray(Wk, dtype=np.float32)
    Wv = np.asarray(Wv, dtype=np.float32)
    Wo = np.asarray(Wo, dtype=np.float32)
    bq = np.asarray(bq, dtype=np.float32)
    bk = np.asarray(bk, dtype=np.float32)
    bv = np.asarray(bv, dtype=np.float32)
    bo = np.asarray(bo, dtype=np.float32)

    x2d = query.reshape(BS, HID)
    xT_bf = np.ascontiguousarray(x2d.T).astype(BF)

    scale = 1.0 / np.sqrt(np.float32(DH))
    in_maps = []
    for c in range(NCORES):
        rows = slice(c * F, (c + 1) * F)
        in_maps.append({
            "xT": xT_bf,
            "wqT": np.ascontiguousarray((Wq[rows] * scale).T).astype(BF),
            "wkT": np.ascontiguousarray(Wk[rows].T).astype(BF),
            "wvT": np.ascontiguousarray(Wv[rows].T).astype(BF),
            "woT": np.ascontiguousarray(Wo[:, rows].T).astype(BF),
            "bq": (bq[rows] * scale).reshape(F, 1).astype(np.float32),
            "bk": bk[rows].reshape(F, 1).astype(np.float32),
            "bv": bv[rows].reshape(F, 1).astype(np.float32),
        })

    nc = _get_nc()
    res = run_bass_kernel_spmd(nc, in_maps, core_ids=list(range(NCORES)))

    attn = np.concatenate([res.results[c]["attn_o"] for c in range(NCORES)],
                          axis=1)
    out = res.results[0]["out_p"].astype(np.float64)
    for c in range(1, NCORES):
        out += res.results[c]["out_p"]
    out = (out + bo).astype(np.float32).reshape(B, S, HID)
    return out, attn


# revision 4
# speedup vs baseline: 1.9657x; 1.9657x over previous
"""Trainium2 Bass kernel for nn_MultiHeadAttention (B=2, S=2048, H=1024, 16 heads).

Sharding: tensor-parallel over heads — 2 heads per core on 8 cores.
Per core: Q/K/V projections for its 128 features, dual score matmuls
(S[i,j] for the attention-weights output + softmax sums via accum_out;
S.T[j,i] recomputed so exp(S.T) feeds the context matmul as the moving
operand — avoids PE transposes entirely), context as ctx.T per head,
out-projection per head with the softmax normalization folded into the
final per-partition combine.  Host gathers attn heads and sums partials.

Self-contained: hardcodes shapes; needs only concourse + numpy + ml_dtypes.
"""
import numpy as np
import ml_dtypes

import concourse.bacc as bacc
import concourse.tile as tile
import concourse.mybir as mybir
from concourse.bass import ts
from concourse.bass_utils import run_bass_kernel_spmd
from concourse.masks import make_identity

B, S, HID = 2, 2048, 1024
NCORES, NH, DH = 8, 16, 64
HPC = NH // NCORES          # 2 heads per core
F = HPC * DH                # 128 features per core
BS = B * S                  # 4096
NI = S // 128               # 16 row chunks per batch
NJ = S // 128               # 16 col chunks per batch
NC8 = HID // 128            # 8 contraction chunks for projections

BF16 = mybir.dt.bfloat16
F32 = mybir.dt.float32
AF = mybir.ActivationFunctionType
ALU = mybir.AluOpType
BF = ml_dtypes.bfloat16


def build_nc():
    nc = bacc.Bacc("TRN2", target_bir_lowering=False, debug=False,
                   num_devices=NCORES)
    xT = nc.dram_tensor("xT", [HID, BS], BF16, kind="ExternalInput")
    wqT = nc.dram_tensor("wqT", [HID, F], BF16, kind="ExternalInput")
    wkT = nc.dram_tensor("wkT", [HID, F], BF16, kind="ExternalInput")
    wvT = nc.dram_tensor("wvT", [HID, F], BF16, kind="ExternalInput")
    woT = nc.dram_tensor("woT", [F, HID], BF16, kind="ExternalInput")
    bq = nc.dram_tensor("bq", [F, 1], F32, kind="ExternalInput")
    bk = nc.dram_tensor("bk", [F, 1], F32, kind="ExternalInput")
    bv = nc.dram_tensor("bv", [F, 1], F32, kind="ExternalInput")
    attn_o = nc.dram_tensor("attn_o", [B, HPC, S, S], F32, kind="ExternalOutput")
    out_p = nc.dram_tensor("out_p", [BS, HID], F32, kind="ExternalOutput")

    with tile.TileContext(nc) as tc:
        with tc.tile_pool(name="consts", bufs=1) as consts, \
             tc.tile_pool(name="p1", bufs=2, space="PSUM") as p1, \
             tc.tile_pool(name="p2", bufs=2, space="PSUM") as p2, \
             tc.tile_pool(name="sb_eS", bufs=3) as sb_eS, \
             tc.tile_pool(name="sb_eT", bufs=3) as sb_eT, \
             tc.tile_pool(name="sb_at", bufs=3) as sb_at, \
             tc.tile_pool(name="sb_ct", bufs=3) as sb_ct, \
             tc.tile_pool(name="sb_out", bufs=3) as sb_out, \
             tc.tile_pool(name="sb_small", bufs=16) as sb_small:

            ident = consts.tile([128, 128], BF16)
            make_identity(nc, ident[:])

            # ---- load inputs ----
            xT_sb = consts.tile([128, NC8, BS], BF16)
            nc.sync.dma_start(out=xT_sb[:],
                              in_=xT[:, :].rearrange("(c p) i -> p c i", p=128))
            wq_sb = consts.tile([128, NC8, F], BF16)
            nc.sync.dma_start(out=wq_sb[:],
                              in_=wqT[:, :].rearrange("(c p) f -> p c f", p=128))
            wk_sb = consts.tile([128, NC8, F], BF16)
            nc.sync.dma_start(out=wk_sb[:],
                              in_=wkT[:, :].rearrange("(c p) f -> p c f", p=128))
            wv_sb = consts.tile([128, NC8, F], BF16)
            nc.sync.dma_start(out=wv_sb[:],
                              in_=wvT[:, :].rearrange("(c p) f -> p c f", p=128))
            # per-head slices of Wo^T rows, each loaded at partition base 0
            wo_h = []
            for lh in range(HPC):
                w = consts.tile([DH, HID], BF16, tag=f"wo{lh}")
                nc.sync.dma_start(out=w[:, :], in_=woT[lh * DH:(lh + 1) * DH, :])
                wo_h.append(w)
            bq_sb = consts.tile([128, 1], F32)
            nc.sync.dma_start(out=bq_sb[:], in_=bq[:, :])
            bk_sb = consts.tile([128, 1], F32)
            nc.sync.dma_start(out=bk_sb[:], in_=bk[:, :])
            bv_sb = consts.tile([128, 1], F32)
            nc.sync.dma_start(out=bv_sb[:], in_=bv[:, :])

            # ---- projections: qT/kT/vT [f=128, i=4096] bf16 with bias ----
            qT_sb = consts.tile([128, BS], BF16)
            kT_sb = consts.tile([128, BS], BF16)
            vT_sb = consts.tile([128, BS], BF16)
            for w_sb, b_sb, dest in ((wq_sb, bq_sb, qT_sb),
                                     (wk_sb, bk_sb, kT_sb),
                                     (wv_sb, bv_sb, vT_sb)):
                for ib in range(BS // 512):
                    ps = p1.tile([128, 512], F32, tag="p1")
                    for cc in range(NC8):
                        nc.tensor.matmul(ps[:],
                                         lhsT=w_sb[:, cc, :],
                                         rhs=xT_sb[:, cc, ts(ib, 512)],
                                         start=(cc == 0), stop=(cc == NC8 - 1))
                    nc.vector.tensor_scalar_add(dest[:, ts(ib, 512)], ps[:], b_sb[:])

            # ---- V into [j, f] layout via PE transpose ----
            v_all = consts.tile([128, B, NJ, F], BF16)
            for b in range(B):
                for jc in range(NJ):
                    pt = p2.tile([128, 128], BF16, tag="p2")
                    nc.tensor.transpose(pt[:], vT_sb[:, b * S + jc * 128:
                                                     b * S + (jc + 1) * 128], ident[:])
                    nc.scalar.copy(v_all[:, b, jc, :], pt[:])

            recip_all = consts.tile([128, HPC, NI], F32)

            # ---- main: per (b, lh): interleaved S.T+context and S phases ----
            for b in range(B):
                ctxT = []  # per-head unnormalized ctx.T [64, 2048] bf16
                for lh in range(HPC):
                    d0 = lh * DH
                    ct_h = [p2.tile([64, 1024], F32, tag="p2", name=f"ct_{b}_{lh}_{ih}")
                            for ih in range(2)]
                    for step in range(NJ):
                        # --- S.T for j-chunk `step`: [j=128, i=2048] ---
                        jc = step
                        eT = sb_eT.tile([128, S], BF16, tag="eT")
                        for ih in range(2):
                            ps = p1.tile([128, 1024], F32, tag="p1")
                            for iq in range(2):
                                i0 = b * S + ih * 1024 + iq * 512
                                nc.tensor.matmul(
                                    ps[:, ts(iq, 512)],
                                    lhsT=kT_sb[d0:d0 + DH,
                                               b * S + jc * 128:b * S + (jc + 1) * 128],
                                    rhs=qT_sb[d0:d0 + DH, i0:i0 + 512],
                                    start=True, stop=True)
                            nc.scalar.activation(out=eT[:, ts(ih, 1024)], in_=ps[:],
                                                 func=AF.Exp)
                        # --- context accumulation: ct[d, i] += V_jc^T-style ---
                        for ih in range(2):
                            for iq in range(2):
                                nc.tensor.matmul(
                                    ct_h[ih][:, ts(iq, 512)],
                                    lhsT=v_all[:, b, jc, d0:d0 + DH],
                                    rhs=eT[:, ih * 1024 + iq * 512:
                                           ih * 1024 + (iq + 1) * 512],
                                    start=(jc == 0), stop=(jc == NJ - 1))
                        # --- S for i-chunk `step`: [i=128, j=2048] ---
                        ic = step
                        i0 = b * S + ic * 128
                        eS = sb_eS.tile([128, S], BF16, tag="eS")
                        sume = sb_small.tile([128, 2], F32, tag="s2")
                        for jh in range(2):
                            ps = p1.tile([128, 1024], F32, tag="p1")
                            for jq in range(2):
                                j0 = b * S + jh * 1024 + jq * 512
                                nc.tensor.matmul(
                                    ps[:, ts(jq, 512)],
                                    lhsT=qT_sb[d0:d0 + DH, i0:i0 + 128],
                                    rhs=kT_sb[d0:d0 + DH, j0:j0 + 512],
                                    start=True, stop=True)
                            nc.scalar.activation(out=eS[:, ts(jh, 1024)], in_=ps[:],
                                                 func=AF.Exp,
                                                 accum_out=sume[:, jh:jh + 1])
                        sumx = sb_small.tile([128, 1], F32, tag="s1")
                        nc.vector.tensor_add(sumx[:], sume[:, 0:1], sume[:, 1:2])
                        nc.vector.reciprocal(recip_all[:, lh, ic:ic + 1], sumx[:])
                        # attn tile normalize (f32 staging) on DVE
                        at = sb_at.tile([128, S], F32, tag="at")
                        nc.vector.tensor_scalar_mul(at[:], eS[:],
                                                    recip_all[:, lh, ic:ic + 1])
                        nc.sync.dma_start(
                            out=attn_o[b, lh, ic * 128:(ic + 1) * 128, :],
                            in_=at[:])
                    # ct -> SBUF bf16 (unnormalized ctx.T)
                    cT = sb_ct.tile([64, S], BF16, tag="ct")
                    for ih in range(2):
                        nc.vector.tensor_copy(cT[:, ts(ih, 1024)], ct_h[ih][:])
                    ctxT.append(cT)

                # ---- out-proj for batch b, fold in 1/sumexp per head ----
                for ic in range(NI):
                    i0 = b * S + ic * 128
                    po = []
                    for lh in range(HPC):
                        p = p1.tile([128, 1024], F32, tag="p1")
                        for oh in range(2):
                            nc.tensor.matmul(
                                p[:, ts(oh, 512)],
                                lhsT=ctxT[lh][:, ic * 128:(ic + 1) * 128],
                                rhs=wo_h[lh][:, ts(oh, 512)],
                                start=True, stop=True)
                        po.append(p)
                    ot = sb_out.tile([128, 1024], F32, tag="o")
                    tmp = sb_out.tile([128, 1024], F32, tag="tmp")
                    nc.vector.tensor_scalar_mul(tmp[:], po[0][:],
                                                recip_all[:, 0, ic:ic + 1])
                    nc.vector.scalar_tensor_tensor(ot[:], po[1][:],
                                                   recip_all[:, 1, ic:ic + 1],
                                                   tmp[:],
                                                   ALU.mult, ALU.add)
                    nc.sync.dma_start(out=out_p[i0:i0 + 128, :], in_=ot[:])

    nc.finalize()
    return nc


_NC = None


def _get_nc():
    global _NC
    if _NC is None:
        _NC = build_nc()
    return _NC


def kernel(query, Wq, bq, Wk, bk, Wv, bv, Wo, bo):
    query = np.asarray(query, dtype=np.float32)
    Wq = np.asarray(Wq, dtype=np.float32)
    Wk = np.asarray(Wk, dtype=np.float32)
    Wv = np.asarray(Wv, dtype=np.float32)
    Wo = np.asarray(Wo, dtype=np.float32)
    bq = np.asarray(bq, dtype=np.float32)
    bk = np.asarray(bk, dtype=np.float32)
    bv = np.asarray(bv, dtype=np.float32)
    bo = np.asarray(bo, dtype=np.float32)

    x2d = query.reshape(BS, HID)
    xT_bf = np.ascontiguousarray(x2d.T).astype(BF)

    scale = 1.0 / np.sqrt(np.float32(DH))
    in_maps = []
    for c in range(NCORES):
        rows = slice(c * F, (c + 1) * F)
        in_maps.append({
            "xT": xT_bf,
            "wqT": np.ascontiguousarray((Wq[rows] * scale).T).astype(BF),
            "wkT": np.ascontiguousarray(Wk[rows].T).astype(BF),
            "wvT": np.ascontiguousarray(Wv[rows].T).astype(BF),
            "woT": np.ascontiguousarray(Wo[:, rows].T).astype(BF),
            "bq": (bq[rows] * scale).reshape(F, 1).astype(np.float32),
            "bk": bk[rows].reshape(F, 1).astype(np.float32),
            "bv": bv[rows].reshape(F, 1).astype(np.float32),
        })

    nc = _get_nc()
    res = run_bass_kernel_spmd(nc, in_maps, core_ids=list(range(NCORES)))

    attn = np.concatenate([res.results[c]["attn_o"] for c in range(NCORES)],
                          axis=1)
    out = res.results[0]["out_p"].astype(np.float64)
    for c in range(1, NCORES):
        out += res.results[c]["out_p"]
    out = (out + bo).astype(np.float32).reshape(B, S, HID)
    return out, attn
